# revision 37
# baseline (speedup 1.0000x reference)
"""Trainium2 Bass kernel for a pre-norm transformer encoder block.

Problem: B=2, T=2048, C=1024, H=16 heads of 64, GELU FFN (4C), fp32.

Sharding: pure data-parallel over (batch, query-slice): 8 cores, core c
handles batch b=c//4 and query rows [(c%4)*512, (c%4+1)*512). Each core
recomputes LN1 + K/V projections for its full batch element (T=2048) so
no cross-core communication is needed; Q/attention/FFN run only on the
core's 512 query rows. All matmul operands are bf16 (fp32 PSUM
accumulation); LN/softmax/residual arithmetic stays fp32.

v1 changes vs baseline: K^T and ones-augmented V stay resident in SBUF
(no DRAM round-trip between projection and attention), LN uses
bn_stats/bn_aggr, the softmax denominator path is
partition_broadcast -> reciprocal_approx_fast (wide tiles), PSUM pools
rebalanced, and phase-C weights prefetch during attention.
"""

import sys

sys.path.insert(0, "/opt/trn_rl_repo")

import numpy as np

import concourse.bass as bass
import concourse.mybir as mybir
import concourse.tile as tile
from concourse import bacc, bass_utils
from concourse.masks import make_identity

P = 128
B, T, C, H = 2, 2048, 1024, 16
HS = C // H  # 64
F = 4 * C  # 4096
NQ = 512  # query rows per core
CC = C // P  # 8
FC = F // P  # 32
TT = T // P  # 16
EPS = 1e-5

f32 = mybir.dt.float32
bfh = mybir.dt.bfloat16
fp8 = mybir.dt.float8e4
AF = mybir.ActivationFunctionType
Alu = mybir.AluOpType
DR = mybir.MatmulPerfMode.DoubleRow

# fp8 power-of-2 scaling: weights |w|<=1/32 (w2: 1/64) scaled into e4m3's
# normal range; activations x16 (folded into LN rsqrt); OT x64 (folded into
# the softmax ones-column = 1/64). Descale via eviction `scale=` immediates.
KW = 12
KW2 = 13
KA = 4
KO = 6
SC_QKV = 2.0 ** (-(KW + KA))
SC_P = 2.0 ** (-(KW + KO))
SC_F1 = 2.0 ** (-(KW + KA))
SC_F2 = 2.0 ** (-KW2)


def _ln_rows(nc, lnp, xt, eps_t):
    """LayerNorm stats for xt [P, C] fp32 -> normalized bf16 tile [P, C]."""
    st6 = lnp.tile([P, 2, 6], f32, tag="ln_st6")
    xv = xt[:].rearrange("p (n f) -> p n f", n=2)
    nc.vector.bn_stats(st6[:, 0, :], xv[:, 0, :])
    nc.vector.bn_stats(st6[:, 1, :], xv[:, 1, :])
    mv = lnp.tile([P, 2], f32, tag="ln_mv")
    nc.vector.bn_aggr(mv[:], st6[:])
    # sd = sqrt(var+eps)/16 so rs = 16/std: xn comes out pre-scaled x16 for fp8
    sd = lnp.tile([P, 1], f32, tag="ln_sd")
    nc.scalar.activation(
        sd[:], mv[:, 1:2], AF.Sqrt, bias=eps_t[:], scale=2.0 ** (-2 * KA)
    )
    rs = lnp.tile([P, 1], f32, tag="ln_rs")
    nc.vector.reciprocal(rs[:], sd[:])
    nmr = lnp.tile([P, 1], f32, tag="ln_nmr")
    nc.vector.tensor_tensor(nmr[:], mv[:, 0:1], rs[:], op=Alu.mult)
    xn = lnp.tile([P, C], bfh, tag="ln_xn")
    nc.vector.tensor_scalar(
        xn[:], xt[:], rs[:], nmr[:], op0=Alu.mult, op1=Alu.subtract
    )
    return xn


def _ln_transpose_block(
    nc, lnp, trp, ident_h, src_ap, dst_bf, dst8, lnw, lnb, eps_t, ntiles=4
):
    """LayerNorm `ntiles`*128 token rows of src_ap [ntiles*128, C] and write
    the transposed, (lnw, lnb)-scaled result into dst_bf [128, CC, ntiles*128]
    (feature-major, bfh), then bulk-convert to dst8 (fp8) on ACT (DVE fp8
    output conversion is not reliable)."""
    for tt in range(ntiles):
        xt = lnp.tile([P, C], f32, tag="ln_x")
        nc.sync.dma_start(xt[:], src_ap[tt * P : (tt + 1) * P, :])
        xn = _ln_rows(nc, lnp, xt, eps_t)
        for cc in range(CC):
            pt = trp.tile([P, P], bfh, tag="ln_tr")
            nc.tensor.transpose(pt[:], xn[:, cc * P : (cc + 1) * P], ident_h[:])
            nc.vector.tensor_scalar(
                dst_bf[:, cc, tt * P : (tt + 1) * P],
                pt[:],
                lnw[:, cc : cc + 1],
                lnb[:, cc : cc + 1],
                op0=Alu.mult,
                op1=Alu.add,
            )
    nc.scalar.copy(dst8[:], dst_bf[:])


def build_program():
    nc = bacc.Bacc("TRN2", target_bir_lowering=False, debug=False, num_devices=8)

    xb_d = nc.dram_tensor("xb", [T, C], f32, kind="ExternalInput").ap()
    xq_d = nc.dram_tensor("xq", [NQ, C], f32, kind="ExternalInput").ap()
    wq_d = nc.dram_tensor("wq", [C, C], fp8, kind="ExternalInput").ap()
    wk_d = nc.dram_tensor("wk", [C, C], fp8, kind="ExternalInput").ap()
    wv_d = nc.dram_tensor("wv", [C, C], fp8, kind="ExternalInput").ap()
    wp_d = nc.dram_tensor("wp", [C, C], fp8, kind="ExternalInput").ap()
    w1_d = nc.dram_tensor("w1", [C, F], fp8, kind="ExternalInput").ap()
    w2_d = nc.dram_tensor("w2", [F, C], fp8, kind="ExternalInput").ap()
    bias_names = ["bq", "bk", "bv", "bp", "b2", "l1w", "l1b", "l2w", "l2b"]
    bias_d = {
        n: nc.dram_tensor(n, [C], f32, kind="ExternalInput").ap() for n in bias_names
    }
    b1_d = nc.dram_tensor("b1", [F], f32, kind="ExternalInput").ap()
    y_d = nc.dram_tensor("y", [NQ, C], f32, kind="ExternalOutput").ap()

    with tile.TileContext(nc) as tc:
        from contextlib import ExitStack

        with ExitStack() as top:
            const = top.enter_context(tc.tile_pool(name="const", bufs=1))
            ident = const.tile([P, P], f32)
            make_identity(nc, ident[:])
            ident_h = const.tile([P, P], bfh, tag="ident_h")
            make_identity(nc, ident_h[:])
            eps_t = const.tile([P, 1], f32, tag="eps")
            nc.vector.memset(eps_t[:], EPS * 2.0 ** (-2 * KA))
            bias_t = {}
            for n in bias_names:
                bt = const.tile([P, CC], f32, tag=f"bias_{n}")
                nc.sync.dma_start(bt[:], bias_d[n].rearrange("(o p) -> p o", p=P))
                bias_t[n] = bt
            b1_t = const.tile([P, FC], f32, tag="bias_b1")
            nc.sync.dma_start(b1_t[:], b1_d.rearrange("(o p) -> p o", p=P))

            # OT lives from B into C
            resOT = top.enter_context(tc.tile_pool(name="resOT", bufs=1))
            OT_t = resOT.tile([P, CC, NQ], fp8, tag="OT")
            # phase-C weights (loaded during attention; pools created here to
            # keep pool release LIFO-ordered)
            wpp = top.enter_context(tc.tile_pool(name="wpp", bufs=1))
            xqp = top.enter_context(tc.tile_pool(name="xqp", bufs=1))

            with ExitStack() as ab:  # pools spanning phases A+B
                resAB = ab.enter_context(tc.tile_pool(name="resAB", bufs=1))
                QT_t = resAB.tile([P, CC, NQ], bfh, tag="QT")
                kT_sb = resAB.tile([P, CC, T], bfh, tag="kT_sb")
                # per (head-pair, k-block): [v_half0 | 1 | v_half1 | 1]
                va_sb = resAB.tile([P, CC, TT, 2 * (HS + 1)], bfh, tag="va_sb")
                # ones columns for the softmax-denominator trick; value 1/64
                # makes the reciprocal come out as 64/d so OT absorbs the
                # x64 fp8 scale for free
                nc.vector.memset(va_sb[:, :, :, HS : HS + 1], 2.0 ** (-KO))
                nc.vector.memset(
                    va_sb[:, :, :, 2 * HS + 1 : 2 * HS + 2], 2.0 ** (-KO)
                )

                # ------------- Phase A: LN1 + Q/K/V projections -------------
                with ExitStack() as ph:
                    lnp = ph.enter_context(tc.tile_pool(name="lnp", bufs=2))
                    trp = ph.enter_context(
                        tc.tile_pool(name="trp", bufs=3, space="PSUM")
                    )
                    mmp = ph.enter_context(
                        tc.tile_pool(name="mmpA", bufs=4, space="PSUM")
                    )
                    xnp = ph.enter_context(tc.tile_pool(name="xnp", bufs=2))
                    wkvp = ph.enter_context(tc.tile_pool(name="wkvp", bufs=1))
                    wqp = ph.enter_context(tc.tile_pool(name="wqp", bufs=2))

                    wk_t = wkvp.tile([P, CC, C], fp8, tag="wk")
                    nc.sync.dma_start(wk_t[:], wk_d.rearrange("(o p) f -> p o f", p=P))
                    wv_t = wkvp.tile([P, CC, C], fp8, tag="wv")
                    nc.sync.dma_start(wv_t[:], wv_d.rearrange("(o p) f -> p o f", p=P))

                    # Q projection from the core's own query slice
                    xnq = xnp.tile([P, CC, NQ], fp8, tag="xnT")
                    xnb = xnp.tile([P, CC, NQ], bfh, tag="xnTb")
                    _ln_transpose_block(
                        nc, lnp, trp, ident_h, xq_d, xnb, xnq,
                        bias_t["l1w"], bias_t["l1b"], eps_t,
                    )
                    for fc in range(CC):
                        wqc = wqp.tile([P, CC, P], fp8, tag="wqc")
                        nc.sync.dma_start(
                            wqc[:],
                            wq_d[:, fc * P : (fc + 1) * P].rearrange(
                                "(o p) f -> p o f", p=P
                            ),
                        )
                        pm = mmp.tile([P, NQ], f32, tag="mmA")
                        for cp in range(CC // 2):
                            nc.tensor.matmul(
                                pm[:],
                                wqc[:, 2 * cp : 2 * cp + 2, :],
                                xnq[:, 2 * cp : 2 * cp + 2, :],
                                start=(cp == 0),
                                stop=(cp == CC // 2 - 1),
                                perf_mode=DR,
                            )
                        nc.vector.tensor_scalar(
                            QT_t[:, fc, :], pm[:], SC_QKV,
                            bias_t["bq"][:, fc : fc + 1],
                            op0=Alu.mult, op1=Alu.add,
                        )

                    # K^T and V over the full batch element, in t-blocks of 512
                    for tb in range(4):
                        xnT = xnp.tile([P, CC, 512], fp8, tag="xnT")
                        xnTb = xnp.tile([P, CC, 512], bfh, tag="xnTb")
                        _ln_transpose_block(
                            nc, lnp, trp, ident_h,
                            xb_d[tb * 512 : (tb + 1) * 512, :],
                            xnTb, xnT, bias_t["l1w"], bias_t["l1b"], eps_t,
                        )
                        for fc in range(CC):
                            pm = mmp.tile([P, 512], f32, tag="mmA")
                            for cp in range(CC // 2):
                                nc.tensor.matmul(
                                    pm[:],
                                    wk_t[:, 2 * cp : 2 * cp + 2, fc * P : (fc + 1) * P],
                                    xnT[:, 2 * cp : 2 * cp + 2, :],
                                    start=(cp == 0),
                                    stop=(cp == CC // 2 - 1),
                                    perf_mode=DR,
                                )
                            nc.vector.tensor_scalar(
                                kT_sb[:, fc, tb * 512 : (tb + 1) * 512], pm[:],
                                SC_QKV, bias_t["bk"][:, fc : fc + 1],
                                op0=Alu.mult, op1=Alu.add,
                            )
                        for fb in range(2):
                            for tt in range(4):
                                pm = mmp.tile([P, 512], f32, tag="mmA")
                                for cp in range(CC // 2):
                                    nc.tensor.matmul(
                                        pm[:],
                                        xnT[
                                            :, 2 * cp : 2 * cp + 2,
                                            tt * P : (tt + 1) * P,
                                        ],
                                        wv_t[
                                            :, 2 * cp : 2 * cp + 2,
                                            fb * 512 : (fb + 1) * 512,
                                        ],
                                        start=(cp == 0),
                                        stop=(cp == CC // 2 - 1),
                                        perf_mode=DR,
                                    )
                                pmv = pm[:].rearrange("p (a h d) -> p a h d", a=4, h=2)
                                nc.vector.tensor_scalar_mul(
                                    va_sb[:, 4 * fb : 4 * fb + 4, tb * 4 + tt, 0:HS],
                                    pmv[:, :, 0, :], SC_QKV,
                                )
                                nc.vector.tensor_scalar_mul(
                                    va_sb[
                                        :, 4 * fb : 4 * fb + 4, tb * 4 + tt,
                                        HS + 1 : 2 * HS + 1,
                                    ],
                                    pmv[:, :, 1, :], SC_QKV,
                                )

                # prefetch phase-C weights during attention
                wp_t = wpp.tile([P, CC, C], fp8, tag="wp")
                nc.sync.dma_start(wp_t[:], wp_d.rearrange("(o p) f -> p o f", p=P))
                xq_t = xqp.tile([P, 4, C], f32, tag="xqt")
                nc.sync.dma_start(xq_t[:], xq_d.rearrange("(q p) c -> p q c", p=P))

                # ------------- Phase B: attention (per head pair) -------------
                with ExitStack() as ph:
                    ep = ph.enter_context(tc.tile_pool(name="ep", bufs=4))
                    sp = ph.enter_context(
                        tc.tile_pool(name="sp", bufs=2, space="PSUM")
                    )
                    op_ = ph.enter_context(
                        tc.tile_pool(name="op", bufs=4, space="PSUM")
                    )
                    npool = ph.enter_context(tc.tile_pool(name="npool", bufs=4))
                    obfp = ph.enter_context(tc.tile_pool(name="obfp", bufs=2))

                    for fc in range(CC):  # head pair (2*fc, 2*fc+1)
                        O0 = op_.tile([P, NQ], f32, tag="Oacc")
                        O1 = op_.tile([P, NQ], f32, tag="Oacc")
                        SKEW = 2
                        pend = {}
                        for step in range(TT + SKEW):
                            if step < TT:
                                tc_i = step
                                s01 = sp.tile([P, 2, NQ], f32, tag="sc")
                                nc.tensor.matmul(
                                    s01[:, 0, :],
                                    kT_sb[0:64, fc, tc_i * P : (tc_i + 1) * P],
                                    QT_t[0:64, fc, :],
                                    start=True,
                                    stop=True,
                                )
                                nc.tensor.matmul(
                                    s01[:, 1, :],
                                    kT_sb[64:128, fc, tc_i * P : (tc_i + 1) * P],
                                    QT_t[64:128, fc, :],
                                    start=True,
                                    stop=True,
                                    tile_position=(64, 0),
                                )
                                e01 = ep.tile([P, 2, NQ], bfh, tag="e01")
                                nc.scalar.activation(
                                    e01[:, 0, :], s01[:, 0, :], AF.Exp, scale=C**-0.5
                                )
                                nc.scalar.activation(
                                    e01[:, 1, :], s01[:, 1, :], AF.Exp, scale=C**-0.5
                                )
                                pend[tc_i] = e01
                            if step >= SKEW:
                                tc_j = step - SKEW
                                e01 = pend.pop(tc_j)
                                nc.tensor.matmul(
                                    O0[0:65, :],
                                    va_sb[:, fc, tc_j, 0 : HS + 1],
                                    e01[:, 0, :],
                                    start=(tc_j == 0), stop=(tc_j == TT - 1),
                                )
                                nc.tensor.matmul(
                                    O1[0:65, :],
                                    va_sb[:, fc, tc_j, HS + 1 : 2 * (HS + 1)],
                                    e01[:, 1, :],
                                    start=(tc_j == 0), stop=(tc_j == TT - 1),
                                )
                        ob = obfp.tile([P, NQ], bfh, tag="ob")
                        for Oacc, col0 in ((O0, 0), (O1, 64)):
                            dn = npool.tile([1, NQ], f32, tag="dn")
                            nc.scalar.copy(dn[:], Oacc[64:65, :])
                            rb = npool.tile([64, NQ], f32, tag="rb")
                            nc.gpsimd.partition_broadcast(
                                rb[:], dn[:], channels=64
                            )
                            rs_ = npool.tile([64, NQ], f32, tag="rsb")
                            nc.vector.reciprocal_approx_fast(rs_[:], rb[:])
                            ot = npool.tile([64, NQ], bfh, tag="otmp")
                            nc.vector.tensor_tensor(
                                ot[:], Oacc[0:64, :], rs_[:], op=Alu.mult
                            )
                            nc.vector.tensor_scalar_add(
                                ob[col0 : col0 + 64, :], ot[:],
                                bias_t["bv"][col0 : col0 + 64, fc : fc + 1],
                            )
                        nc.scalar.copy(OT_t[:, fc, :], ob[:])

            # ------------- Phase C: out-proj + residual + LN2 -------------
            resC = top.enter_context(tc.tile_pool(name="resC", bufs=1))
            outq_t = resC.tile([P, 4, C], f32, tag="outq")
            onT_t = resC.tile([P, CC, NQ], fp8, tag="onT")
            with ExitStack() as ph:
                lnp = ph.enter_context(tc.tile_pool(name="lnpC", bufs=2))
                trp = ph.enter_context(tc.tile_pool(name="trpC", bufs=3, space="PSUM"))
                mmp = ph.enter_context(tc.tile_pool(name="mmpC", bufs=3, space="PSUM"))
                evp = ph.enter_context(tc.tile_pool(name="evpC", bufs=3))

                for co in range(CC):
                    pm = mmp.tile([P, NQ], f32, tag="mmC")
                    for ci in range(CC // 2):
                        nc.tensor.matmul(
                            pm[:],
                            wp_t[:, 2 * ci : 2 * ci + 2, co * P : (co + 1) * P],
                            OT_t[:, 2 * ci : 2 * ci + 2, :],
                            start=(ci == 0),
                            stop=(ci == CC // 2 - 1),
                            perf_mode=DR,
                        )
                    saT = evp.tile([P, NQ], f32, tag="saT")
                    nc.vector.tensor_scalar(
                        saT[:], pm[:], SC_P, bias_t["bp"][:, co : co + 1],
                        op0=Alu.mult, op1=Alu.add,
                    )
                    for qt in range(4):
                        pt = trp.tile([P, P], f32, tag="trC")
                        nc.tensor.transpose(
                            pt[:], saT[:, qt * P : (qt + 1) * P], ident[:]
                        )
                        nc.vector.tensor_tensor(
                            outq_t[:, qt, co * P : (co + 1) * P],
                            pt[:],
                            xq_t[:, qt, co * P : (co + 1) * P],
                            op=Alu.add,
                        )
                # LN2 (token-major, input already in SBUF) -> feature-major onT
                onb = resC.tile([P, CC, NQ], bfh, tag="onTb")
                for qt in range(4):
                    xn = _ln_rows(nc, lnp, outq_t[:, qt, :], eps_t)
                    for cc in range(CC):
                        pt = trp.tile([P, P], bfh, tag="trC")
                        nc.tensor.transpose(
                            pt[:], xn[:, cc * P : (cc + 1) * P], ident_h[:]
                        )
                        nc.vector.tensor_scalar(
                            onb[:, cc, qt * P : (qt + 1) * P],
                            pt[:],
                            bias_t["l2w"][:, cc : cc + 1],
                            bias_t["l2b"][:, cc : cc + 1],
                            op0=Alu.mult,
                            op1=Alu.add,
                        )
                nc.scalar.copy(onT_t[:], onb[:])

            # ---------------- Phase D: FFN ----------------
            with ExitStack() as ph:
                w1p = ph.enter_context(tc.tile_pool(name="w1p", bufs=3))
                w2p = ph.enter_context(tc.tile_pool(name="w2p", bufs=2))
                hp = ph.enter_context(tc.tile_pool(name="hp", bufs=1))
                mmph = ph.enter_context(tc.tile_pool(name="mmph", bufs=3, space="PSUM"))
                mmpy = ph.enter_context(tc.tile_pool(name="mmpy", bufs=2, space="PSUM"))
                trp = ph.enter_context(tc.tile_pool(name="trpD", bufs=2, space="PSUM"))
                evp = ph.enter_context(tc.tile_pool(name="evpD", bufs=3))
                finp = ph.enter_context(tc.tile_pool(name="finp", bufs=1))

                hT_t = hp.tile([P, FC, NQ], fp8, tag="hT")
                final_t = finp.tile([P, 4, C], f32, tag="final")

                for fc in range(FC):
                    w1c = w1p.tile([P, CC, P], fp8, tag="w1c")
                    nc.sync.dma_start(
                        w1c[:],
                        w1_d[:, fc * P : (fc + 1) * P].rearrange(
                            "(o p) f -> p o f", p=P
                        ),
                    )
                    pm = mmph.tile([P, NQ], f32, tag="mmh")
                    for cp in range(CC // 2):
                        nc.tensor.matmul(
                            pm[:],
                            w1c[:, 2 * cp : 2 * cp + 2, :],
                            onT_t[:, 2 * cp : 2 * cp + 2, :],
                            start=(cp == 0),
                            stop=(cp == CC // 2 - 1),
                            perf_mode=DR,
                        )
                    nc.scalar.activation(
                        hT_t[:, fc, :], pm[:], AF.Gelu,
                        bias=b1_t[:, fc : fc + 1], scale=SC_F1,
                    )

                for co in range(CC):
                    w2c = w2p.tile([P, FC, P], fp8, tag="w2c")
                    nc.sync.dma_start(
                        w2c[:],
                        w2_d[:, co * P : (co + 1) * P].rearrange(
                            "(o p) f -> p o f", p=P
                        ),
                    )
                    pm = mmpy.tile([P, NQ], f32, tag="mmy")
                    for fp_ in range(FC // 2):
                        nc.tensor.matmul(
                            pm[:],
                            w2c[:, 2 * fp_ : 2 * fp_ + 2, :],
                            hT_t[:, 2 * fp_ : 2 * fp_ + 2, :],
                            start=(fp_ == 0),
                            stop=(fp_ == FC // 2 - 1),
                            perf_mode=DR,
                        )
                    yT = evp.tile([P, NQ], f32, tag="yT")
                    nc.vector.tensor_scalar(
                        yT[:], pm[:], SC_F2, bias_t["b2"][:, co : co + 1],
                        op0=Alu.mult, op1=Alu.add,
                    )
                    for qt in range(4):
                        pt = trp.tile([P, P], f32, tag="trD")
                        nc.tensor.transpose(
                            pt[:], yT[:, qt * P : (qt + 1) * P], ident[:]
                        )
                        nc.vector.tensor_tensor(
                            final_t[:, qt, co * P : (co + 1) * P],
                            pt[:],
                            outq_t[:, qt, co * P : (co + 1) * P],
                            op=Alu.add,
                        )
                nc.sync.dma_start(
                    y_d.rearrange("(q p) c -> p q c", p=P), final_t[:]
                )

    nc.compile()
    return nc


_NC_CACHE = None


def _get_program():
    global _NC_CACHE
    if _NC_CACHE is None:
        _NC_CACHE = build_program()
    return _NC_CACHE


import ml_dtypes

BF16 = ml_dtypes.bfloat16
FP8 = ml_dtypes.float8_e4m3fn


def _merge_heads(w):
    # [H, C, HS] -> [C, H*HS]
    return np.ascontiguousarray(
        np.transpose(np.asarray(w), (1, 0, 2)).reshape(C, C).astype(np.float32)
    )


def _q8(w, k):
    w = np.asarray(w, np.float32) * (2.0**k)
    return np.ascontiguousarray(np.clip(w, -192.0, 192.0).astype(FP8))


def make_in_maps(inputs):
    x = np.ascontiguousarray(np.asarray(inputs["x"], dtype=np.float32))
    shared = {
        "wq": _q8(_merge_heads(inputs["Wq"]), KW),
        "wk": _q8(_merge_heads(inputs["Wk"]), KW),
        "wv": _q8(_merge_heads(inputs["Wv"]), KW),
        "wp": _q8(inputs["Wp"], KW),
        "w1": _q8(inputs["W1"], KW),
        "w2": _q8(inputs["W2"], KW2),
        "bq": np.asarray(inputs["bq"], np.float32).reshape(C).copy(),
        "bk": np.asarray(inputs["bk"], np.float32).reshape(C).copy(),
        "bv": np.asarray(inputs["bv"], np.float32).reshape(C) * (2.0**KO),
        "bp": np.asarray(inputs["bp"], np.float32).copy(),
        "b1": np.asarray(inputs["b1"], np.float32).copy(),
        "b2": np.asarray(inputs["b2"], np.float32).copy(),
        "l1w": np.asarray(inputs["ln1_w"], np.float32).copy(),
        "l1b": np.asarray(inputs["ln1_b"], np.float32) * (2.0**KA),
        "l2w": np.asarray(inputs["ln2_w"], np.float32).copy(),
        "l2b": np.asarray(inputs["ln2_b"], np.float32) * (2.0**KA),
    }
    in_maps = []
    for c in range(8):
        b, qs = c // 4, c % 4
        m = dict(shared)
        m["xb"] = np.ascontiguousarray(x[b])
        m["xq"] = np.ascontiguousarray(x[b, qs * NQ : (qs + 1) * NQ])
        in_maps.append(m)
    return in_maps


def kernel(**inputs):
    in_maps = make_in_maps(inputs)
    nc = _get_program()
    res = bass_utils.run_bass_kernel_spmd(nc, in_maps, core_ids=list(range(8)))
    out = np.empty((B, T, C), np.float32)
    for c in range(8):
        b, qs = c // 4, c % 4
        out[b, qs * NQ : (qs + 1) * NQ] = res.results[c]["y"]
    return out


# revision 38
# speedup vs baseline: 1.0494x; 1.0494x over previous
"""Trainium2 Bass kernel for a pre-norm transformer encoder block.

Problem: B=2, T=2048, C=1024, H=16 heads of 64, GELU FFN (4C), fp32.

Sharding: pure data-parallel over (batch, query-slice): 8 cores, core c
handles batch b=c//4 and query rows [(c%4)*512, (c%4+1)*512). Each core
recomputes LN1 + K/V projections for its full batch element (T=2048) so
no cross-core communication is needed; Q/attention/FFN run only on the
core's 512 query rows. All matmul operands are bf16 (fp32 PSUM
accumulation); LN/softmax/residual arithmetic stays fp32.

v1 changes vs baseline: K^T and ones-augmented V stay resident in SBUF
(no DRAM round-trip between projection and attention), LN uses
bn_stats/bn_aggr, the softmax denominator path is
partition_broadcast -> reciprocal_approx_fast (wide tiles), PSUM pools
rebalanced, and phase-C weights prefetch during attention.
"""

import sys

sys.path.insert(0, "/opt/trn_rl_repo")

import numpy as np

import concourse.bass as bass
import concourse.mybir as mybir
import concourse.tile as tile
from concourse import bacc, bass_utils
from concourse.masks import make_identity

P = 128
B, T, C, H = 2, 2048, 1024, 16
HS = C // H  # 64
F = 4 * C  # 4096
NQ = 512  # query rows per core
CC = C // P  # 8
FC = F // P  # 32
TT = T // P  # 16
EPS = 1e-5

f32 = mybir.dt.float32
bfh = mybir.dt.bfloat16
fp8 = mybir.dt.float8e4
AF = mybir.ActivationFunctionType
Alu = mybir.AluOpType
DR = mybir.MatmulPerfMode.DoubleRow

# fp8 power-of-2 scaling: weights |w|<=1/32 (w2: 1/64) scaled into e4m3's
# normal range; activations x16 (folded into LN rsqrt); OT x64 (folded into
# the softmax ones-column = 1/64). Descale via eviction `scale=` immediates.
KW = 12
KW2 = 13
KA = 4
KO = 6
SC_QKV = 2.0 ** (-(KW + KA))
SC_P = 2.0 ** (-(KW + KO))
SC_F1 = 2.0 ** (-(KW + KA))
SC_F2 = 2.0 ** (-KW2)


def _ln_rows(nc, lnp, xt, eps_t):
    """LayerNorm stats for xt [P, C] fp32 -> normalized bf16 tile [P, C]."""
    st6 = lnp.tile([P, 2, 6], f32, tag="ln_st6")
    xv = xt[:].rearrange("p (n f) -> p n f", n=2)
    nc.vector.bn_stats(st6[:, 0, :], xv[:, 0, :])
    nc.vector.bn_stats(st6[:, 1, :], xv[:, 1, :])
    mv = lnp.tile([P, 2], f32, tag="ln_mv")
    nc.vector.bn_aggr(mv[:], st6[:])
    # sd = sqrt(var+eps)/16 so rs = 16/std: xn comes out pre-scaled x16 for fp8
    sd = lnp.tile([P, 1], f32, tag="ln_sd")
    nc.scalar.activation(
        sd[:], mv[:, 1:2], AF.Sqrt, bias=eps_t[:], scale=2.0 ** (-2 * KA)
    )
    rs = lnp.tile([P, 1], f32, tag="ln_rs")
    nc.vector.reciprocal(rs[:], sd[:])
    nmr = lnp.tile([P, 1], f32, tag="ln_nmr")
    nc.vector.tensor_tensor(nmr[:], mv[:, 0:1], rs[:], op=Alu.mult)
    xn = lnp.tile([P, C], bfh, tag="ln_xn")
    nc.vector.tensor_scalar(
        xn[:], xt[:], rs[:], nmr[:], op0=Alu.mult, op1=Alu.subtract
    )
    return xn


def _ln_transpose_block(
    nc, lnp, trp, ident_h, src_ap, dst_bf, dst8, lnw, lnb, eps_t, ntiles=4
):
    """LayerNorm `ntiles`*128 token rows of src_ap [ntiles*128, C] and write
    the transposed, (lnw, lnb)-scaled result into dst_bf [128, CC, ntiles*128]
    (feature-major, bfh), then bulk-convert to dst8 (fp8) on ACT (DVE fp8
    output conversion is not reliable)."""
    for tt in range(ntiles):
        xt = lnp.tile([P, C], f32, tag="ln_x")
        nc.sync.dma_start(xt[:], src_ap[tt * P : (tt + 1) * P, :])
        xn = _ln_rows(nc, lnp, xt, eps_t)
        for cc in range(CC):
            pt = trp.tile([P, P], bfh, tag="ln_tr")
            nc.tensor.transpose(pt[:], xn[:, cc * P : (cc + 1) * P], ident_h[:])
            nc.vector.tensor_scalar(
                dst_bf[:, cc, tt * P : (tt + 1) * P],
                pt[:],
                lnw[:, cc : cc + 1],
                lnb[:, cc : cc + 1],
                op0=Alu.mult,
                op1=Alu.add,
            )
    nc.scalar.copy(dst8[:], dst_bf[:])


def build_program():
    nc = bacc.Bacc("TRN2", target_bir_lowering=False, debug=False, num_devices=8)

    xb_d = nc.dram_tensor("xb", [T, C], f32, kind="ExternalInput").ap()
    xq_d = nc.dram_tensor("xq", [NQ, C], f32, kind="ExternalInput").ap()
    wq_d = nc.dram_tensor("wq", [C, C], fp8, kind="ExternalInput").ap()
    wk_d = nc.dram_tensor("wk", [C, C], fp8, kind="ExternalInput").ap()
    wv_d = nc.dram_tensor("wv", [C, C], fp8, kind="ExternalInput").ap()
    wp_d = nc.dram_tensor("wp", [C, C], fp8, kind="ExternalInput").ap()
    w1_d = nc.dram_tensor("w1", [C, F], fp8, kind="ExternalInput").ap()
    w2_d = nc.dram_tensor("w2", [F, C], fp8, kind="ExternalInput").ap()
    bias_names = ["bq", "bk", "bv", "bp", "b2", "l1w", "l1b", "l2w", "l2b"]
    bias_d = {
        n: nc.dram_tensor(n, [C], f32, kind="ExternalInput").ap() for n in bias_names
    }
    b1_d = nc.dram_tensor("b1", [F], f32, kind="ExternalInput").ap()
    y_d = nc.dram_tensor("y", [NQ, C], f32, kind="ExternalOutput").ap()

    with tile.TileContext(nc) as tc:
        from contextlib import ExitStack

        with ExitStack() as top:
            const = top.enter_context(tc.tile_pool(name="const", bufs=1))
            ident = const.tile([P, P], f32)
            make_identity(nc, ident[:])
            ident_h = const.tile([P, P], bfh, tag="ident_h")
            make_identity(nc, ident_h[:])
            eps_t = const.tile([P, 1], f32, tag="eps")
            nc.vector.memset(eps_t[:], EPS * 2.0 ** (-2 * KA))
            bias_t = {}
            for n in bias_names:
                bt = const.tile([P, CC], f32, tag=f"bias_{n}")
                nc.sync.dma_start(bt[:], bias_d[n].rearrange("(o p) -> p o", p=P))
                bias_t[n] = bt
            b1_t = const.tile([P, FC], f32, tag="bias_b1")
            nc.sync.dma_start(b1_t[:], b1_d.rearrange("(o p) -> p o", p=P))

            # OT lives from B into C
            resOT = top.enter_context(tc.tile_pool(name="resOT", bufs=1))
            OT_t = resOT.tile([P, CC, NQ], fp8, tag="OT")
            # phase-C weights (loaded during attention; pools created here to
            # keep pool release LIFO-ordered)
            wpp = top.enter_context(tc.tile_pool(name="wpp", bufs=1))
            xqp = top.enter_context(tc.tile_pool(name="xqp", bufs=1))

            with ExitStack() as ab:  # pools spanning phases A+B
                resAB = ab.enter_context(tc.tile_pool(name="resAB", bufs=1))
                QT_t = resAB.tile([P, CC, NQ], bfh, tag="QT")
                kT_sb = resAB.tile([P, CC, T], bfh, tag="kT_sb")
                # per (head-pair, k-block): [v_half0 | 1 | v_half1 | 1]
                va_sb = resAB.tile([P, CC, TT, 2 * (HS + 1)], bfh, tag="va_sb")
                # ones columns for the softmax-denominator trick; value 1/64
                # makes the reciprocal come out as 64/d so OT absorbs the
                # x64 fp8 scale for free
                nc.vector.memset(va_sb[:, :, :, HS : HS + 1], 2.0 ** (-KO))
                nc.vector.memset(
                    va_sb[:, :, :, 2 * HS + 1 : 2 * HS + 2], 2.0 ** (-KO)
                )

                # ------------- Phase A: LN1 + Q/K/V projections -------------
                with ExitStack() as ph:
                    lnp = ph.enter_context(tc.tile_pool(name="lnp", bufs=2))
                    trp = ph.enter_context(
                        tc.tile_pool(name="trp", bufs=3, space="PSUM")
                    )
                    mmp = ph.enter_context(
                        tc.tile_pool(name="mmpA", bufs=4, space="PSUM")
                    )
                    xnp = ph.enter_context(tc.tile_pool(name="xnp", bufs=2))
                    wkvp = ph.enter_context(tc.tile_pool(name="wkvp", bufs=1))
                    wqp = ph.enter_context(tc.tile_pool(name="wqp", bufs=2))

                    wk_t = wkvp.tile([P, CC, C], fp8, tag="wk")
                    nc.sync.dma_start(wk_t[:], wk_d.rearrange("(o p) f -> p o f", p=P))
                    wv_t = wkvp.tile([P, CC, C], fp8, tag="wv")
                    nc.sync.dma_start(wv_t[:], wv_d.rearrange("(o p) f -> p o f", p=P))

                    # Q projection from the core's own query slice
                    xnq = xnp.tile([P, CC, NQ], fp8, tag="xnT")
                    xnb = xnp.tile([P, CC, NQ], bfh, tag="xnTb")
                    _ln_transpose_block(
                        nc, lnp, trp, ident_h, xq_d, xnb, xnq,
                        bias_t["l1w"], bias_t["l1b"], eps_t,
                    )
                    for fc in range(CC):
                        wqc = wqp.tile([P, CC, P], fp8, tag="wqc")
                        nc.sync.dma_start(
                            wqc[:],
                            wq_d[:, fc * P : (fc + 1) * P].rearrange(
                                "(o p) f -> p o f", p=P
                            ),
                        )
                        pm = mmp.tile([P, NQ], f32, tag="mmA")
                        for cp in range(CC // 2):
                            nc.tensor.matmul(
                                pm[:],
                                wqc[:, 2 * cp : 2 * cp + 2, :],
                                xnq[:, 2 * cp : 2 * cp + 2, :],
                                start=(cp == 0),
                                stop=(cp == CC // 2 - 1),
                                perf_mode=DR,
                            )
                        nc.scalar.activation(
                            QT_t[:, fc, :], pm[:], AF.Identity,
                            bias=bias_t["bq"][:, fc : fc + 1], scale=SC_QKV,
                        )

                    # K^T and V over the full batch element, in t-blocks of 512
                    for tb in range(4):
                        xnT = xnp.tile([P, CC, 512], fp8, tag="xnT")
                        xnTb = xnp.tile([P, CC, 512], bfh, tag="xnTb")
                        _ln_transpose_block(
                            nc, lnp, trp, ident_h,
                            xb_d[tb * 512 : (tb + 1) * 512, :],
                            xnTb, xnT, bias_t["l1w"], bias_t["l1b"], eps_t,
                        )
                        for fc in range(CC):
                            pm = mmp.tile([P, 512], f32, tag="mmA")
                            for cp in range(CC // 2):
                                nc.tensor.matmul(
                                    pm[:],
                                    wk_t[:, 2 * cp : 2 * cp + 2, fc * P : (fc + 1) * P],
                                    xnT[:, 2 * cp : 2 * cp + 2, :],
                                    start=(cp == 0),
                                    stop=(cp == CC // 2 - 1),
                                    perf_mode=DR,
                                )
                            nc.scalar.activation(
                                kT_sb[:, fc, tb * 512 : (tb + 1) * 512], pm[:],
                                AF.Identity, bias=bias_t["bk"][:, fc : fc + 1],
                                scale=SC_QKV,
                            )
                        for fb in range(2):
                            for tt in range(4):
                                pm = mmp.tile([P, 512], f32, tag="mmA")
                                for cp in range(CC // 2):
                                    nc.tensor.matmul(
                                        pm[:],
                                        xnT[
                                            :, 2 * cp : 2 * cp + 2,
                                            tt * P : (tt + 1) * P,
                                        ],
                                        wv_t[
                                            :, 2 * cp : 2 * cp + 2,
                                            fb * 512 : (fb + 1) * 512,
                                        ],
                                        start=(cp == 0),
                                        stop=(cp == CC // 2 - 1),
                                        perf_mode=DR,
                                    )
                                pmv = pm[:].rearrange("p (a h d) -> p a h d", a=4, h=2)
                                nc.scalar.activation(
                                    va_sb[:, 4 * fb : 4 * fb + 4, tb * 4 + tt, 0:HS],
                                    pmv[:, :, 0, :], AF.Identity, scale=SC_QKV,
                                )
                                nc.scalar.activation(
                                    va_sb[
                                        :, 4 * fb : 4 * fb + 4, tb * 4 + tt,
                                        HS + 1 : 2 * HS + 1,
                                    ],
                                    pmv[:, :, 1, :], AF.Identity, scale=SC_QKV,
                                )

                # prefetch phase-C weights during attention
                wp_t = wpp.tile([P, CC, C], fp8, tag="wp")
                nc.sync.dma_start(wp_t[:], wp_d.rearrange("(o p) f -> p o f", p=P))
                xq_t = xqp.tile([P, 4, C], f32, tag="xqt")
                nc.sync.dma_start(xq_t[:], xq_d.rearrange("(q p) c -> p q c", p=P))

                # ------------- Phase B: attention (per head pair) -------------
                with ExitStack() as ph:
                    ep = ph.enter_context(tc.tile_pool(name="ep", bufs=4))
                    sp = ph.enter_context(
                        tc.tile_pool(name="sp", bufs=2, space="PSUM")
                    )
                    op_ = ph.enter_context(
                        tc.tile_pool(name="op", bufs=4, space="PSUM")
                    )
                    npool = ph.enter_context(tc.tile_pool(name="npool", bufs=4))
                    obfp = ph.enter_context(tc.tile_pool(name="obfp", bufs=2))

                    for fc in range(CC):  # head pair (2*fc, 2*fc+1)
                        O0 = op_.tile([P, NQ], f32, tag="Oacc")
                        O1 = op_.tile([P, NQ], f32, tag="Oacc")
                        SKEW = 2
                        pend = {}
                        for step in range(TT + SKEW):
                            if step < TT:
                                tc_i = step
                                s01 = sp.tile([P, 2, NQ], f32, tag="sc")
                                nc.tensor.matmul(
                                    s01[:, 0, :],
                                    kT_sb[0:64, fc, tc_i * P : (tc_i + 1) * P],
                                    QT_t[0:64, fc, :],
                                    start=True,
                                    stop=True,
                                )
                                nc.tensor.matmul(
                                    s01[:, 1, :],
                                    kT_sb[64:128, fc, tc_i * P : (tc_i + 1) * P],
                                    QT_t[64:128, fc, :],
                                    start=True,
                                    stop=True,
                                    tile_position=(64, 0),
                                )
                                e01 = ep.tile([P, 2, NQ], bfh, tag="e01")
                                nc.scalar.activation(
                                    e01[:, 0, :], s01[:, 0, :], AF.Exp, scale=C**-0.5
                                )
                                nc.scalar.activation(
                                    e01[:, 1, :], s01[:, 1, :], AF.Exp, scale=C**-0.5
                                )
                                pend[tc_i] = e01
                            if step >= SKEW:
                                tc_j = step - SKEW
                                e01 = pend.pop(tc_j)
                                nc.tensor.matmul(
                                    O0[0:65, :],
                                    va_sb[:, fc, tc_j, 0 : HS + 1],
                                    e01[:, 0, :],
                                    start=(tc_j == 0), stop=(tc_j == TT - 1),
                                )
                                nc.tensor.matmul(
                                    O1[0:65, :],
                                    va_sb[:, fc, tc_j, HS + 1 : 2 * (HS + 1)],
                                    e01[:, 1, :],
                                    start=(tc_j == 0), stop=(tc_j == TT - 1),
                                )
                        ob = obfp.tile([P, NQ], bfh, tag="ob")
                        for Oacc, col0 in ((O0, 0), (O1, 64)):
                            dn = npool.tile([1, NQ], f32, tag="dn")
                            nc.scalar.copy(dn[:], Oacc[64:65, :])
                            rb = npool.tile([64, NQ], f32, tag="rb")
                            nc.gpsimd.partition_broadcast(
                                rb[:], dn[:], channels=64
                            )
                            rs_ = npool.tile([64, NQ], f32, tag="rsb")
                            nc.vector.reciprocal_approx_fast(rs_[:], rb[:])
                            ot = npool.tile([64, NQ], bfh, tag="otmp")
                            nc.vector.tensor_tensor(
                                ot[:], Oacc[0:64, :], rs_[:], op=Alu.mult
                            )
                            nc.vector.tensor_scalar_add(
                                ob[col0 : col0 + 64, :], ot[:],
                                bias_t["bv"][col0 : col0 + 64, fc : fc + 1],
                            )
                        nc.scalar.copy(OT_t[:, fc, :], ob[:])

            # ------------- Phase C: out-proj + residual + LN2 -------------
            resC = top.enter_context(tc.tile_pool(name="resC", bufs=1))
            outq_t = resC.tile([P, 4, C], f32, tag="outq")
            onT_t = resC.tile([P, CC, NQ], fp8, tag="onT")
            with ExitStack() as ph:
                lnp = ph.enter_context(tc.tile_pool(name="lnpC", bufs=2))
                trp = ph.enter_context(tc.tile_pool(name="trpC", bufs=3, space="PSUM"))
                mmp = ph.enter_context(tc.tile_pool(name="mmpC", bufs=3, space="PSUM"))
                evp = ph.enter_context(tc.tile_pool(name="evpC", bufs=3))

                for co in range(CC):
                    pm = mmp.tile([P, NQ], f32, tag="mmC")
                    for ci in range(CC // 2):
                        nc.tensor.matmul(
                            pm[:],
                            wp_t[:, 2 * ci : 2 * ci + 2, co * P : (co + 1) * P],
                            OT_t[:, 2 * ci : 2 * ci + 2, :],
                            start=(ci == 0),
                            stop=(ci == CC // 2 - 1),
                            perf_mode=DR,
                        )
                    saT = evp.tile([P, NQ], f32, tag="saT")
                    nc.scalar.activation(
                        saT[:], pm[:], AF.Identity,
                        bias=bias_t["bp"][:, co : co + 1], scale=SC_P,
                    )
                    for qt in range(4):
                        pt = trp.tile([P, P], f32, tag="trC")
                        nc.tensor.transpose(
                            pt[:], saT[:, qt * P : (qt + 1) * P], ident[:]
                        )
                        nc.vector.tensor_tensor(
                            outq_t[:, qt, co * P : (co + 1) * P],
                            pt[:],
                            xq_t[:, qt, co * P : (co + 1) * P],
                            op=Alu.add,
                        )
                # LN2 (token-major, input already in SBUF) -> feature-major onT
                onb = resC.tile([P, CC, NQ], bfh, tag="onTb")
                for qt in range(4):
                    xn = _ln_rows(nc, lnp, outq_t[:, qt, :], eps_t)
                    for cc in range(CC):
                        pt = trp.tile([P, P], bfh, tag="trC")
                        nc.tensor.transpose(
                            pt[:], xn[:, cc * P : (cc + 1) * P], ident_h[:]
                        )
                        nc.vector.tensor_scalar(
                            onb[:, cc, qt * P : (qt + 1) * P],
                            pt[:],
                            bias_t["l2w"][:, cc : cc + 1],
                            bias_t["l2b"][:, cc : cc + 1],
                            op0=Alu.mult,
                            op1=Alu.add,
                        )
                nc.scalar.copy(onT_t[:], onb[:])

            # ---------------- Phase D: FFN ----------------
            with ExitStack() as ph:
                w1p = ph.enter_context(tc.tile_pool(name="w1p", bufs=3))
                w2p = ph.enter_context(tc.tile_pool(name="w2p", bufs=2))
                hp = ph.enter_context(tc.tile_pool(name="hp", bufs=1))
                mmph = ph.enter_context(tc.tile_pool(name="mmph", bufs=3, space="PSUM"))
                mmpy = ph.enter_context(tc.tile_pool(name="mmpy", bufs=2, space="PSUM"))
                trp = ph.enter_context(tc.tile_pool(name="trpD", bufs=2, space="PSUM"))
                evp = ph.enter_context(tc.tile_pool(name="evpD", bufs=3))
                finp = ph.enter_context(tc.tile_pool(name="finp", bufs=1))

                hT_t = hp.tile([P, FC, NQ], fp8, tag="hT")
                final_t = finp.tile([P, 4, C], f32, tag="final")

                for fc in range(FC):
                    w1c = w1p.tile([P, CC, P], fp8, tag="w1c")
                    nc.sync.dma_start(
                        w1c[:],
                        w1_d[:, fc * P : (fc + 1) * P].rearrange(
                            "(o p) f -> p o f", p=P
                        ),
                    )
                    pm = mmph.tile([P, NQ], f32, tag="mmh")
                    for cp in range(CC // 2):
                        nc.tensor.matmul(
                            pm[:],
                            w1c[:, 2 * cp : 2 * cp + 2, :],
                            onT_t[:, 2 * cp : 2 * cp + 2, :],
                            start=(cp == 0),
                            stop=(cp == CC // 2 - 1),
                            perf_mode=DR,
                        )
                    nc.scalar.activation(
                        hT_t[:, fc, :], pm[:], AF.Gelu,
                        bias=b1_t[:, fc : fc + 1], scale=SC_F1,
                    )

                for co in range(CC):
                    w2c = w2p.tile([P, FC, P], fp8, tag="w2c")
                    nc.sync.dma_start(
                        w2c[:],
                        w2_d[:, co * P : (co + 1) * P].rearrange(
                            "(o p) f -> p o f", p=P
                        ),
                    )
                    pm = mmpy.tile([P, NQ], f32, tag="mmy")
                    for fp_ in range(FC // 2):
                        nc.tensor.matmul(
                            pm[:],
                            w2c[:, 2 * fp_ : 2 * fp_ + 2, :],
                            hT_t[:, 2 * fp_ : 2 * fp_ + 2, :],
                            start=(fp_ == 0),
                            stop=(fp_ == FC // 2 - 1),
                            perf_mode=DR,
                        )
                    yT = evp.tile([P, NQ], f32, tag="yT")
                    nc.scalar.activation(
                        yT[:], pm[:], AF.Identity,
                        bias=bias_t["b2"][:, co : co + 1], scale=SC_F2,
                    )
                    for qt in range(4):
                        pt = trp.tile([P, P], f32, tag="trD")
                        nc.tensor.transpose(
                            pt[:], yT[:, qt * P : (qt + 1) * P], ident[:]
                        )
                        nc.vector.tensor_tensor(
                            final_t[:, qt, co * P : (co + 1) * P],
                            pt[:],
                            outq_t[:, qt, co * P : (co + 1) * P],
                            op=Alu.add,
                        )
                nc.sync.dma_start(
                    y_d.rearrange("(q p) c -> p q c", p=P), final_t[:]
                )

    nc.compile()
    return nc


_NC_CACHE = None


def _get_program():
    global _NC_CACHE
    if _NC_CACHE is None:
        _NC_CACHE = build_program()
    return _NC_CACHE


import ml_dtypes

BF16 = ml_dtypes.bfloat16
FP8 = ml_dtypes.float8_e4m3fn


def _merge_heads(w):
    # [H, C, HS] -> [C, H*HS]
    return np.ascontiguousarray(
        np.transpose(np.asarray(w), (1, 0, 2)).reshape(C, C).astype(np.float32)
    )


def _q8(w, k):
    w = np.asarray(w, np.float32) * (2.0**k)
    return np.ascontiguousarray(np.clip(w, -192.0, 192.0).astype(FP8))


def make_in_maps(inputs):
    x = np.ascontiguousarray(np.asarray(inputs["x"], dtype=np.float32))
    shared = {
        "wq": _q8(_merge_heads(inputs["Wq"]), KW),
        "wk": _q8(_merge_heads(inputs["Wk"]), KW),
        "wv": _q8(_merge_heads(inputs["Wv"]), KW),
        "wp": _q8(inputs["Wp"], KW),
        "w1": _q8(inputs["W1"], KW),
        "w2": _q8(inputs["W2"], KW2),
        "bq": np.asarray(inputs["bq"], np.float32).reshape(C).copy(),
        "bk": np.asarray(inputs["bk"], np.float32).reshape(C).copy(),
        "bv": np.asarray(inputs["bv"], np.float32).reshape(C) * (2.0**KO),
        "bp": np.asarray(inputs["bp"], np.float32).copy(),
        "b1": np.asarray(inputs["b1"], np.float32).copy(),
        "b2": np.asarray(inputs["b2"], np.float32).copy(),
        "l1w": np.asarray(inputs["ln1_w"], np.float32).copy(),
        "l1b": np.asarray(inputs["ln1_b"], np.float32) * (2.0**KA),
        "l2w": np.asarray(inputs["ln2_w"], np.float32).copy(),
        "l2b": np.asarray(inputs["ln2_b"], np.float32) * (2.0**KA),
    }
    in_maps = []
    for c in range(8):
        b, qs = c // 4, c % 4
        m = dict(shared)
        m["xb"] = np.ascontiguousarray(x[b])
        m["xq"] = np.ascontiguousarray(x[b, qs * NQ : (qs + 1) * NQ])
        in_maps.append(m)
    return in_maps


def kernel(**inputs):
    in_maps = make_in_maps(inputs)
    nc = _get_program()
    res = bass_utils.run_bass_kernel_spmd(nc, in_maps, core_ids=list(range(8)))
    out = np.empty((B, T, C), np.float32)
    for c in range(8):
        b, qs = c // 4, c % 4
        out[b, qs * NQ : (qs + 1) * NQ] = res.results[c]["y"]
    return out


# revision 40
# speedup vs baseline: 1.1037x; 1.0518x over previous
"""Trainium2 Bass kernel for a pre-norm transformer encoder block.

Problem: B=2, T=2048, C=1024, H=16 heads of 64, GELU FFN (4C), fp32.

Sharding: pure data-parallel over (batch, query-slice): 8 cores, core c
handles batch b=c//4 and query rows [(c%4)*512, (c%4+1)*512). Each core
recomputes LN1 + K/V projections for its full batch element (T=2048) so
no cross-core communication is needed; Q/attention/FFN run only on the
core's 512 query rows. Projection/FFN matmuls run in fp8e4 with
DoubleRow perf mode (2 k-tiles per matmul); attention scores/attV stay
bf16; LN/softmax/residual arithmetic stays fp32.

vs baseline: K^T and ones-augmented V stay resident in SBUF (no DRAM
round-trip between projection and attention), LN uses bn_stats/bn_aggr,
the softmax denominator path is partition_broadcast ->
reciprocal_approx_fast, fp8 DoubleRow halves projection matmul count
(power-of-2 weight/activation scales folded into LN rsqrt, the
softmax ones-column, and eviction scale immediates), PSUM pools
rebalanced, and phase-C weights prefetch during attention.
"""

import sys

sys.path.insert(0, "/opt/trn_rl_repo")

import numpy as np

import concourse.bass as bass
import concourse.mybir as mybir
import concourse.tile as tile
from concourse import bacc, bass_utils
from concourse.masks import make_identity

P = 128
B, T, C, H = 2, 2048, 1024, 16
HS = C // H  # 64
F = 4 * C  # 4096
NQ = 512  # query rows per core
CC = C // P  # 8
FC = F // P  # 32
TT = T // P  # 16
EPS = 1e-5

f32 = mybir.dt.float32
bfh = mybir.dt.bfloat16
fp8 = mybir.dt.float8e4
AF = mybir.ActivationFunctionType
Alu = mybir.AluOpType
DR = mybir.MatmulPerfMode.DoubleRow

# fp8 power-of-2 scaling: weights |w|<=1/32 (w2: 1/64) scaled into e4m3's
# normal range; activations x16 (folded into LN rsqrt); OT x64 (folded into
# the softmax ones-column = 1/64). Descale via eviction `scale=` immediates.
KW = 12
KW2 = 13
KA = 4
KO = 6
SC_QKV = 2.0 ** (-(KW + KA))
SC_P = 2.0 ** (-(KW + KO))
SC_F1 = 2.0 ** (-(KW + KA))
SC_F2 = 2.0 ** (-KW2)


def _ln_rows(nc, lnp, xt, eps_t):
    """LayerNorm stats for xt [P, C] fp32 -> normalized bf16 tile [P, C]."""
    st6 = lnp.tile([P, 2, 6], f32, tag="ln_st6")
    xv = xt[:].rearrange("p (n f) -> p n f", n=2)
    nc.vector.bn_stats(st6[:, 0, :], xv[:, 0, :])
    nc.vector.bn_stats(st6[:, 1, :], xv[:, 1, :])
    mv = lnp.tile([P, 2], f32, tag="ln_mv")
    nc.vector.bn_aggr(mv[:], st6[:])
    # sd = sqrt(var+eps)/16 so rs = 16/std: xn comes out pre-scaled x16 for fp8
    sd = lnp.tile([P, 1], f32, tag="ln_sd")
    nc.scalar.activation(
        sd[:], mv[:, 1:2], AF.Sqrt, bias=eps_t[:], scale=2.0 ** (-2 * KA)
    )
    rs = lnp.tile([P, 1], f32, tag="ln_rs")
    nc.vector.reciprocal(rs[:], sd[:])
    nmr = lnp.tile([P, 1], f32, tag="ln_nmr")
    nc.vector.tensor_tensor(nmr[:], mv[:, 0:1], rs[:], op=Alu.mult)
    xn = lnp.tile([P, C], bfh, tag="ln_xn")
    nc.vector.tensor_scalar(
        xn[:], xt[:], rs[:], nmr[:], op0=Alu.mult, op1=Alu.subtract
    )
    return xn


def _ln_transpose_block(
    nc, lnp, trp, ident_h, src_ap, dst_bf, dst8, lnw, lnb, eps_t, ntiles=4
):
    """LayerNorm `ntiles`*128 token rows of src_ap [ntiles*128, C] and write
    the transposed, (lnw, lnb)-scaled result into dst_bf [128, CC, ntiles*128]
    (feature-major, bfh), then bulk-convert to dst8 (fp8) on ACT (DVE fp8
    output conversion is not reliable)."""
    for tt in range(ntiles):
        xt = lnp.tile([P, C], f32, tag="ln_x")
        nc.sync.dma_start(xt[:], src_ap[tt * P : (tt + 1) * P, :])
        xn = _ln_rows(nc, lnp, xt, eps_t)
        for cc in range(CC):
            pt = trp.tile([P, P], bfh, tag="ln_tr")
            nc.tensor.transpose(pt[:], xn[:, cc * P : (cc + 1) * P], ident_h[:])
            nc.vector.tensor_scalar(
                dst_bf[:, cc, tt * P : (tt + 1) * P],
                pt[:],
                lnw[:, cc : cc + 1],
                lnb[:, cc : cc + 1],
                op0=Alu.mult,
                op1=Alu.add,
            )
    nc.scalar.copy(dst8[:], dst_bf[:])


def build_program():
    nc = bacc.Bacc("TRN2", target_bir_lowering=False, debug=False, num_devices=8)

    xb_d = nc.dram_tensor("xb", [T, C], f32, kind="ExternalInput").ap()
    xq_d = nc.dram_tensor("xq", [NQ, C], f32, kind="ExternalInput").ap()
    wq_d = nc.dram_tensor("wq", [C, C], fp8, kind="ExternalInput").ap()
    wk_d = nc.dram_tensor("wk", [C, C], fp8, kind="ExternalInput").ap()
    wv_d = nc.dram_tensor("wv", [C, C], fp8, kind="ExternalInput").ap()
    wp_d = nc.dram_tensor("wp", [C, C], fp8, kind="ExternalInput").ap()
    w1_d = nc.dram_tensor("w1", [C, F], fp8, kind="ExternalInput").ap()
    w2_d = nc.dram_tensor("w2", [F, C], fp8, kind="ExternalInput").ap()
    bias_names = ["bq", "bk", "bv", "bp", "b2", "l1w", "l1b", "l2w", "l2b"]
    bias_d = {
        n: nc.dram_tensor(n, [C], f32, kind="ExternalInput").ap() for n in bias_names
    }
    b1_d = nc.dram_tensor("b1", [F], f32, kind="ExternalInput").ap()
    y_d = nc.dram_tensor("y", [NQ, C], f32, kind="ExternalOutput").ap()

    with tile.TileContext(nc) as tc:
        from contextlib import ExitStack

        with ExitStack() as top:
            const = top.enter_context(tc.tile_pool(name="const", bufs=1))
            ident = const.tile([P, P], f32)
            make_identity(nc, ident[:])
            ident_h = const.tile([P, P], bfh, tag="ident_h")
            make_identity(nc, ident_h[:])
            eps_t = const.tile([P, 1], f32, tag="eps")
            nc.vector.memset(eps_t[:], EPS * 2.0 ** (-2 * KA))
            bias_t = {}
            for n in bias_names:
                bt = const.tile([P, CC], f32, tag=f"bias_{n}")
                nc.sync.dma_start(bt[:], bias_d[n].rearrange("(o p) -> p o", p=P))
                bias_t[n] = bt
            b1_t = const.tile([P, FC], f32, tag="bias_b1")
            nc.sync.dma_start(b1_t[:], b1_d.rearrange("(o p) -> p o", p=P))

            # OT lives from B into C
            resOT = top.enter_context(tc.tile_pool(name="resOT", bufs=1))
            OT_t = resOT.tile([P, CC, NQ], fp8, tag="OT")
            # phase-C weights (loaded during attention; pools created here to
            # keep pool release LIFO-ordered)
            wpp = top.enter_context(tc.tile_pool(name="wpp", bufs=1))
            xqp = top.enter_context(tc.tile_pool(name="xqp", bufs=1))

            with ExitStack() as ab:  # pools spanning phases A+B
                resAB = ab.enter_context(tc.tile_pool(name="resAB", bufs=1))
                QT_t = resAB.tile([P, CC, NQ], bfh, tag="QT")
                kT_sb = resAB.tile([P, CC, T], bfh, tag="kT_sb")
                # per (head-pair, k-block): [v_half0 | 1 | v_half1 | 1]
                va_sb = resAB.tile([P, CC, TT, 2 * (HS + 1)], bfh, tag="va_sb")
                # ones columns for the softmax-denominator trick; value 1/64
                # makes the reciprocal come out as 64/d so OT absorbs the
                # x64 fp8 scale for free
                nc.vector.memset(va_sb[:, :, :, HS : HS + 1], 2.0 ** (-KO))
                nc.vector.memset(
                    va_sb[:, :, :, 2 * HS + 1 : 2 * HS + 2], 2.0 ** (-KO)
                )

                # ------------- Phase A: LN1 + Q/K/V projections -------------
                with ExitStack() as ph:
                    lnp = ph.enter_context(tc.tile_pool(name="lnp", bufs=2))
                    trp = ph.enter_context(
                        tc.tile_pool(name="trp", bufs=3, space="PSUM")
                    )
                    mmp = ph.enter_context(
                        tc.tile_pool(name="mmpA", bufs=4, space="PSUM")
                    )
                    xnp = ph.enter_context(tc.tile_pool(name="xnp", bufs=2))
                    wkvp = ph.enter_context(tc.tile_pool(name="wkvp", bufs=1))
                    wqp = ph.enter_context(tc.tile_pool(name="wqp", bufs=2))

                    wk_t = wkvp.tile([P, CC, C], fp8, tag="wk")
                    nc.sync.dma_start(wk_t[:], wk_d.rearrange("(o p) f -> p o f", p=P))
                    wv_t = wkvp.tile([P, CC, C], fp8, tag="wv")
                    nc.sync.dma_start(wv_t[:], wv_d.rearrange("(o p) f -> p o f", p=P))

                    # Q projection from the core's own query slice
                    xnq = xnp.tile([P, CC, NQ], fp8, tag="xnT")
                    xnb = xnp.tile([P, CC, NQ], bfh, tag="xnTb")
                    _ln_transpose_block(
                        nc, lnp, trp, ident_h, xq_d, xnb, xnq,
                        bias_t["l1w"], bias_t["l1b"], eps_t,
                    )
                    for fc in range(CC):
                        wqc = wqp.tile([P, CC, P], fp8, tag="wqc")
                        nc.sync.dma_start(
                            wqc[:],
                            wq_d[:, fc * P : (fc + 1) * P].rearrange(
                                "(o p) f -> p o f", p=P
                            ),
                        )
                        pm = mmp.tile([P, NQ], f32, tag="mmA")
                        for cp in range(CC // 2):
                            nc.tensor.matmul(
                                pm[:],
                                wqc[:, 2 * cp : 2 * cp + 2, :],
                                xnq[:, 2 * cp : 2 * cp + 2, :],
                                start=(cp == 0),
                                stop=(cp == CC // 2 - 1),
                                perf_mode=DR,
                            )
                        nc.scalar.activation(
                            QT_t[:, fc, :], pm[:], AF.Identity,
                            bias=bias_t["bq"][:, fc : fc + 1], scale=SC_QKV,
                        )

                    # K^T and V over the full batch element, in t-blocks of 512
                    for tb in range(4):
                        xnT = xnp.tile([P, CC, 512], fp8, tag="xnT")
                        xnTb = xnp.tile([P, CC, 512], bfh, tag="xnTb")
                        _ln_transpose_block(
                            nc, lnp, trp, ident_h,
                            xb_d[tb * 512 : (tb + 1) * 512, :],
                            xnTb, xnT, bias_t["l1w"], bias_t["l1b"], eps_t,
                        )
                        for fc in range(CC):
                            pm = mmp.tile([P, 512], f32, tag="mmA")
                            for cp in range(CC // 2):
                                nc.tensor.matmul(
                                    pm[:],
                                    wk_t[:, 2 * cp : 2 * cp + 2, fc * P : (fc + 1) * P],
                                    xnT[:, 2 * cp : 2 * cp + 2, :],
                                    start=(cp == 0),
                                    stop=(cp == CC // 2 - 1),
                                    perf_mode=DR,
                                )
                            nc.scalar.activation(
                                kT_sb[:, fc, tb * 512 : (tb + 1) * 512], pm[:],
                                AF.Identity, bias=bias_t["bk"][:, fc : fc + 1],
                                scale=SC_QKV,
                            )
                        for fb in range(2):
                            for tt in range(4):
                                pm = mmp.tile([P, 512], f32, tag="mmA")
                                for cp in range(CC // 2):
                                    nc.tensor.matmul(
                                        pm[:],
                                        xnT[
                                            :, 2 * cp : 2 * cp + 2,
                                            tt * P : (tt + 1) * P,
                                        ],
                                        wv_t[
                                            :, 2 * cp : 2 * cp + 2,
                                            fb * 512 : (fb + 1) * 512,
                                        ],
                                        start=(cp == 0),
                                        stop=(cp == CC // 2 - 1),
                                        perf_mode=DR,
                                    )
                                pmv = pm[:].rearrange("p (a h d) -> p a h d", a=4, h=2)
                                nc.scalar.activation(
                                    va_sb[:, 4 * fb : 4 * fb + 4, tb * 4 + tt, 0:HS],
                                    pmv[:, :, 0, :], AF.Identity, scale=SC_QKV,
                                )
                                nc.scalar.activation(
                                    va_sb[
                                        :, 4 * fb : 4 * fb + 4, tb * 4 + tt,
                                        HS + 1 : 2 * HS + 1,
                                    ],
                                    pmv[:, :, 1, :], AF.Identity, scale=SC_QKV,
                                )

                # prefetch phase-C weights during attention
                wp_t = wpp.tile([P, CC, C], fp8, tag="wp")
                nc.sync.dma_start(wp_t[:], wp_d.rearrange("(o p) f -> p o f", p=P))
                xq_t = xqp.tile([P, 4, C], f32, tag="xqt")
                nc.sync.dma_start(xq_t[:], xq_d.rearrange("(q p) c -> p q c", p=P))

                # ------------- Phase B: attention (per head pair) -------------
                with ExitStack() as ph:
                    ep = ph.enter_context(tc.tile_pool(name="ep", bufs=4))
                    sp = ph.enter_context(
                        tc.tile_pool(name="sp", bufs=2, space="PSUM")
                    )
                    op_ = ph.enter_context(
                        tc.tile_pool(name="op", bufs=4, space="PSUM")
                    )
                    npool = ph.enter_context(tc.tile_pool(name="npool", bufs=4))
                    obfp = ph.enter_context(tc.tile_pool(name="obfp", bufs=2))

                    for fc in range(CC):  # head pair (2*fc, 2*fc+1)
                        O0 = op_.tile([P, NQ], f32, tag="Oacc")
                        O1 = op_.tile([P, NQ], f32, tag="Oacc")
                        SKEW = 2
                        pend = {}
                        for step in range(TT + SKEW):
                            if step < TT:
                                tc_i = step
                                s01 = sp.tile([P, 2, NQ], f32, tag="sc")
                                nc.tensor.matmul(
                                    s01[:, 0, :],
                                    kT_sb[0:64, fc, tc_i * P : (tc_i + 1) * P],
                                    QT_t[0:64, fc, :],
                                    start=True,
                                    stop=True,
                                )
                                nc.tensor.matmul(
                                    s01[:, 1, :],
                                    kT_sb[64:128, fc, tc_i * P : (tc_i + 1) * P],
                                    QT_t[64:128, fc, :],
                                    start=True,
                                    stop=True,
                                    tile_position=(64, 0),
                                )
                                e01 = ep.tile([P, 2, NQ], bfh, tag="e01")
                                nc.scalar.activation(
                                    e01[:], s01[:], AF.Exp, scale=C**-0.5
                                )
                                pend[tc_i] = e01
                            if step >= SKEW:
                                tc_j = step - SKEW
                                e01 = pend.pop(tc_j)
                                nc.tensor.matmul(
                                    O0[0:65, :],
                                    va_sb[:, fc, tc_j, 0 : HS + 1],
                                    e01[:, 0, :],
                                    start=(tc_j == 0), stop=(tc_j == TT - 1),
                                )
                                nc.tensor.matmul(
                                    O1[0:65, :],
                                    va_sb[:, fc, tc_j, HS + 1 : 2 * (HS + 1)],
                                    e01[:, 1, :],
                                    start=(tc_j == 0), stop=(tc_j == TT - 1),
                                )
                        ob = obfp.tile([P, NQ], bfh, tag="ob")
                        for Oacc, col0 in ((O0, 0), (O1, 64)):
                            dn = npool.tile([1, NQ], f32, tag="dn")
                            nc.scalar.copy(dn[:], Oacc[64:65, :])
                            rb = npool.tile([64, NQ], f32, tag="rb")
                            nc.gpsimd.partition_broadcast(
                                rb[:], dn[:], channels=64
                            )
                            rs_ = npool.tile([64, NQ], f32, tag="rsb")
                            nc.vector.reciprocal_approx_fast(rs_[:], rb[:])
                            ot = npool.tile([64, NQ], bfh, tag="otmp")
                            nc.vector.tensor_tensor(
                                ot[:], Oacc[0:64, :], rs_[:], op=Alu.mult
                            )
                            nc.vector.tensor_scalar_add(
                                ob[col0 : col0 + 64, :], ot[:],
                                bias_t["bv"][col0 : col0 + 64, fc : fc + 1],
                            )
                        nc.scalar.copy(OT_t[:, fc, :], ob[:])

            # ------------- Phase C: out-proj + residual + LN2 -------------
            resC = top.enter_context(tc.tile_pool(name="resC", bufs=1))
            outq_t = resC.tile([P, 4, C], f32, tag="outq")
            onT_t = resC.tile([P, CC, NQ], fp8, tag="onT")
            with ExitStack() as ph:
                lnp = ph.enter_context(tc.tile_pool(name="lnpC", bufs=2))
                trp = ph.enter_context(tc.tile_pool(name="trpC", bufs=3, space="PSUM"))
                mmp = ph.enter_context(tc.tile_pool(name="mmpC", bufs=3, space="PSUM"))
                evp = ph.enter_context(tc.tile_pool(name="evpC", bufs=3))

                for co in range(CC):
                    pm = mmp.tile([P, NQ], f32, tag="mmC")
                    for ci in range(CC // 2):
                        nc.tensor.matmul(
                            pm[:],
                            wp_t[:, 2 * ci : 2 * ci + 2, co * P : (co + 1) * P],
                            OT_t[:, 2 * ci : 2 * ci + 2, :],
                            start=(ci == 0),
                            stop=(ci == CC // 2 - 1),
                            perf_mode=DR,
                        )
                    saT = evp.tile([P, NQ], f32, tag="saT")
                    nc.scalar.activation(
                        saT[:], pm[:], AF.Identity,
                        bias=bias_t["bp"][:, co : co + 1], scale=SC_P,
                    )
                    for qt in range(4):
                        pt = trp.tile([P, P], f32, tag="trC")
                        nc.tensor.transpose(
                            pt[:], saT[:, qt * P : (qt + 1) * P], ident[:]
                        )
                        nc.vector.tensor_tensor(
                            outq_t[:, qt, co * P : (co + 1) * P],
                            pt[:],
                            xq_t[:, qt, co * P : (co + 1) * P],
                            op=Alu.add,
                        )
                # LN2 (token-major, input already in SBUF) -> feature-major onT
                onb = resC.tile([P, CC, NQ], bfh, tag="onTb")
                for qt in range(4):
                    xn = _ln_rows(nc, lnp, outq_t[:, qt, :], eps_t)
                    for cc in range(CC):
                        pt = trp.tile([P, P], bfh, tag="trC")
                        nc.tensor.transpose(
                            pt[:], xn[:, cc * P : (cc + 1) * P], ident_h[:]
                        )
                        nc.vector.tensor_scalar(
                            onb[:, cc, qt * P : (qt + 1) * P],
                            pt[:],
                            bias_t["l2w"][:, cc : cc + 1],
                            bias_t["l2b"][:, cc : cc + 1],
                            op0=Alu.mult,
                            op1=Alu.add,
                        )
                nc.scalar.copy(onT_t[:], onb[:])

            # ---------------- Phase D: FFN ----------------
            with ExitStack() as ph:
                w1p = ph.enter_context(tc.tile_pool(name="w1p", bufs=3))
                w2p = ph.enter_context(tc.tile_pool(name="w2p", bufs=2))
                hp = ph.enter_context(tc.tile_pool(name="hp", bufs=1))
                mmph = ph.enter_context(tc.tile_pool(name="mmph", bufs=3, space="PSUM"))
                mmpy = ph.enter_context(tc.tile_pool(name="mmpy", bufs=2, space="PSUM"))
                trp = ph.enter_context(tc.tile_pool(name="trpD", bufs=2, space="PSUM"))
                evp = ph.enter_context(tc.tile_pool(name="evpD", bufs=3))
                finp = ph.enter_context(tc.tile_pool(name="finp", bufs=1))

                hT_t = hp.tile([P, FC, NQ], fp8, tag="hT")
                final_t = finp.tile([P, 4, C], f32, tag="final")

                for fc in range(FC):
                    w1c = w1p.tile([P, CC, P], fp8, tag="w1c")
                    nc.sync.dma_start(
                        w1c[:],
                        w1_d[:, fc * P : (fc + 1) * P].rearrange(
                            "(o p) f -> p o f", p=P
                        ),
                    )
                    pm = mmph.tile([P, NQ], f32, tag="mmh")
                    for cp in range(CC // 2):
                        nc.tensor.matmul(
                            pm[:],
                            w1c[:, 2 * cp : 2 * cp + 2, :],
                            onT_t[:, 2 * cp : 2 * cp + 2, :],
                            start=(cp == 0),
                            stop=(cp == CC // 2 - 1),
                            perf_mode=DR,
                        )
                    nc.scalar.activation(
                        hT_t[:, fc, :], pm[:], AF.Gelu,
                        bias=b1_t[:, fc : fc + 1], scale=SC_F1,
                    )

                for co in range(CC):
                    w2c = w2p.tile([P, FC, P], fp8, tag="w2c")
                    nc.sync.dma_start(
                        w2c[:],
                        w2_d[:, co * P : (co + 1) * P].rearrange(
                            "(o p) f -> p o f", p=P
                        ),
                    )
                    pm = mmpy.tile([P, NQ], f32, tag="mmy")
                    for fp_ in range(FC // 2):
                        nc.tensor.matmul(
                            pm[:],
                            w2c[:, 2 * fp_ : 2 * fp_ + 2, :],
                            hT_t[:, 2 * fp_ : 2 * fp_ + 2, :],
                            start=(fp_ == 0),
                            stop=(fp_ == FC // 2 - 1),
                            perf_mode=DR,
                        )
                    yT = evp.tile([P, NQ], f32, tag="yT")
                    nc.scalar.activation(
                        yT[:], pm[:], AF.Identity,
                        bias=bias_t["b2"][:, co : co + 1], scale=SC_F2,
                    )
                    for qt in range(4):
                        pt = trp.tile([P, P], f32, tag="trD")
                        nc.tensor.transpose(
                            pt[:], yT[:, qt * P : (qt + 1) * P], ident[:]
                        )
                        nc.vector.tensor_tensor(
                            final_t[:, qt, co * P : (co + 1) * P],
                            pt[:],
                            outq_t[:, qt, co * P : (co + 1) * P],
                            op=Alu.add,
                        )
                nc.sync.dma_start(
                    y_d.rearrange("(q p) c -> p q c", p=P), final_t[:]
                )

    nc.compile()
    return nc


_NC_CACHE = None


def _get_program():
    global _NC_CACHE
    if _NC_CACHE is None:
        _NC_CACHE = build_program()
    return _NC_CACHE


import ml_dtypes

BF16 = ml_dtypes.bfloat16
FP8 = ml_dtypes.float8_e4m3fn


def _merge_heads(w):
    # [H, C, HS] -> [C, H*HS]
    return np.ascontiguousarray(
        np.transpose(np.asarray(w), (1, 0, 2)).reshape(C, C).astype(np.float32)
    )


def _q8(w, k):
    w = np.asarray(w, np.float32) * (2.0**k)
    return np.ascontiguousarray(np.clip(w, -192.0, 192.0).astype(FP8))


def make_in_maps(inputs):
    x = np.ascontiguousarray(np.asarray(inputs["x"], dtype=np.float32))
    shared = {
        "wq": _q8(_merge_heads(inputs["Wq"]), KW),
        "wk": _q8(_merge_heads(inputs["Wk"]), KW),
        "wv": _q8(_merge_heads(inputs["Wv"]), KW),
        "wp": _q8(inputs["Wp"], KW),
        "w1": _q8(inputs["W1"], KW),
        "w2": _q8(inputs["W2"], KW2),
        "bq": np.asarray(inputs["bq"], np.float32).reshape(C).copy(),
        "bk": np.asarray(inputs["bk"], np.float32).reshape(C).copy(),
        "bv": np.asarray(inputs["bv"], np.float32).reshape(C) * (2.0**KO),
        "bp": np.asarray(inputs["bp"], np.float32).copy(),
        "b1": np.asarray(inputs["b1"], np.float32).copy(),
        "b2": np.asarray(inputs["b2"], np.float32).copy(),
        "l1w": np.asarray(inputs["ln1_w"], np.float32).copy(),
        "l1b": np.asarray(inputs["ln1_b"], np.float32) * (2.0**KA),
        "l2w": np.asarray(inputs["ln2_w"], np.float32).copy(),
        "l2b": np.asarray(inputs["ln2_b"], np.float32) * (2.0**KA),
    }
    in_maps = []
    for c in range(8):
        b, qs = c // 4, c % 4
        m = dict(shared)
        m["xb"] = np.ascontiguousarray(x[b])
        m["xq"] = np.ascontiguousarray(x[b, qs * NQ : (qs + 1) * NQ])
        in_maps.append(m)
    return in_maps


def kernel(**inputs):
    in_maps = make_in_maps(inputs)
    nc = _get_program()
    res = bass_utils.run_bass_kernel_spmd(nc, in_maps, core_ids=list(range(8)))
    out = np.empty((B, T, C), np.float32)
    for c in range(8):
        b, qs = c // 4, c % 4
        out[b, qs * NQ : (qs + 1) * NQ] = res.results[c]["y"]
    return out
